# revision 22
# baseline (speedup 1.0000x reference)
"""Trainium2 Bass kernel for nn_ModelName_86242943303934 (gnn_message_passing).

Self-contained: takes FULL inputs, shards across 8 NeuronCores internally,
runs one SPMD Bass/Tile program, gathers the full [2048, 1] output.

v2: inputs are bit-packed on host (7 bits/byte for the 0/1 incidence
matrices) and expanded on device with a single uint32 AND per 28 output
columns, bitcast to fp8 (each bit position k yields an exact power-of-two
value BITVAL[k], folded into the 1/de and 1/dv normalization scales).
All transposes run on the tensor engine (PSUM) with the per-partition
de/dv scales fused into the PSUM->SBUF copies on the scalar engine.
Segment one-hot matrices and the groupid selector are built on device
from iota + is_equal; choose_emb comes from a 1MB AllGather of the
propagated group embedding instead of host-gathered H_gg rows.
"""
import sys
sys.path.insert(0, '/opt/trn_rl_repo')

import numpy as np
import ml_dtypes

import concourse.bass as bass
import concourse.mybir as mybir
import concourse.tile as tile
from concourse import bacc
from concourse.bass_utils import run_bass_kernel_spmd
from concourse.masks import make_identity

bf16 = ml_dtypes.bfloat16
f8 = ml_dtypes.float8_e4m3fn
FP32 = mybir.dt.float32
BF16 = mybir.dt.bfloat16
F8 = mybir.dt.float8e4
U32 = mybir.dt.uint32
I16 = mybir.dt.int16

NC = 8
U, G, D, B = 30000, 4096, 128, 2048
UC = U // NC            # 3750 local users
KU = 30                 # user chunks of 128 (padded)
UCP = KU * 128          # 3840
USUB = 480              # pass-B u-subtile width (8 * 480 = 3840)
NUS = 8
GGR = G // NC           # 512 local H_gg rows
KG = 4                  # gg chunks of 128
BC = B // NC            # 256 batch rows per core
NGC = 32                # g chunks of 128
GS = 8                  # 512-col slices of G
W512 = 19               # u32 words per 512-col packed slice
W480 = 18               # u32 words per 480-col packed slice
E512 = W512 * 28        # 532 expanded cols per 512-slice
E480 = W480 * 28        # 504 expanded cols per 480-slice

# value of a set bit k when its byte is reinterpreted as fp8e4m3
BITVAL = np.array([2.0**-9, 2.0**-8, 2.0**-7, 2.0**-6, 2.0**-5, 2.0**-3, 2.0],
                  np.float32)

AF = mybir.ActivationFunctionType


def _slotmap(w):
    """Positions m in [0,w) -> (word j, bit k, byte h) for the packing whose
    on-device expansion (AND with masks, fp8 bitcast) restores column order."""
    nfull, rem = divmod(w, 28)
    assert rem % 4 == 0 and rem // 4 <= 7
    m = np.arange(w)
    j = m // 28
    q = m - 28 * j
    k = q // 4
    h = q % 4
    return j, k, h, nfull + (1 if rem else 0)


def _pack(block, w):
    """block [R, w] of 0/1 -> uint32 words [R, nw]."""
    j, k, h, nw = _slotmap(w)
    bits = (block > 0.5).astype(np.uint32) << (8 * h + k).astype(np.uint32)[None, :]
    starts = np.searchsorted(j, np.arange(nw))
    return np.ascontiguousarray(
        np.bitwise_or.reduceat(bits, starts, axis=1).astype(np.uint32))


def _kcol(w):
    _, k, _, _ = _slotmap(w)
    return k


def _pack_rows(Mrows, w):
    """Mrows [R, C] with C % w == 0 -> [R, (C//w)*nw] u32, per-w-slice packed."""
    R, C = Mrows.shape
    ns = C // w
    outs = [_pack(Mrows[:, s * w:(s + 1) * w], w) for s in range(ns)]
    return np.ascontiguousarray(np.concatenate(outs, axis=1))


def _wrap_idx(idx, n):
    cols = (n + 15) // 16
    w = np.zeros((16, cols), np.int16)
    for i in range(n):
        w[i % 16, i // 16] = idx[i]
    return w


def _prep(inputs):
    inp = {k: np.asarray(v) for k, v in inputs.items()}
    H = {'a': inp['H_ug'].astype(np.float32),
         'b': inp['H_ug_affect'].astype(np.float32)}
    Hg = inp['H_gg'].astype(np.float32)
    user_emb = inp['user_emb'].astype(np.float32)
    group_emb = inp['group_emb'].astype(np.float32)
    item_emb = inp['item_emb'].astype(np.float32)
    groupid = inp['groupid'].astype(np.int64)
    itemid = inp['itemid'].astype(np.int64)
    mids = inp['member_user_ids'].astype(np.int64)
    bseg = inp['batch_seg'].astype(np.int64)

    att_w1 = inp['att_w1'].astype(np.float32)
    att_b1 = inp['att_b1'].astype(np.float32)
    att_w2 = inp['att_w2'].astype(np.float32)
    pw1 = inp['pred_w1'].astype(np.float32)
    pb1 = inp['pred_b1'].astype(np.float32)
    pw2 = inp['pred_w2'].astype(np.float32)

    deg = {}
    for m, Hm in (('a', H['a']), ('b', H['b']), ('g', Hg)):
        deg[m] = (Hm.sum(1) + 1e-5, Hm.sum(0) + 1e-5)

    counts = np.bincount(bseg, minlength=B)
    starts = np.concatenate([[0], np.cumsum(counts)])
    mc = [int(starts[(c + 1) * BC] - starts[c * BC]) for c in range(NC)]
    MPAD = int(-(-max(mc) // 256) * 256)
    NJ = MPAD // 128

    item_b = item_emb[itemid]                      # [B, D] host gather

    kg = np.tile(_kcol(512), GS)                   # k-index per g column
    ku = np.tile(_kcol(480), NUS)                  # k-index per local user col
    kgg = _kcol(512)                               # k-index per local gg col

    masks = (np.uint32(0x01010101) << np.arange(7, dtype=np.uint32))
    masks = np.tile(masks[None, :], (128, 1)).astype(np.uint32)

    in_maps = []
    for c in range(NC):
        m = {'masks': masks}
        for key in ('a', 'b'):
            rows = slice(c * UC, (c + 1) * UC)
            Hp = np.zeros((UCP, G), np.float32)
            Hp[:UC] = H[key][rows]
            # pass-a bits: [KU, 128, GS*W512]
            pb = _pack_rows(Hp, 512)               # [UCP, 152]
            m[f'hub_{key}'] = np.ascontiguousarray(
                pb.reshape(KU // 2, 2, 128, GS * W512).transpose(0, 2, 1, 3))
            # pass-b panel bits: per us, [128(part=g%128), NGC*W480]
            HT = Hp.T                              # [G, UCP]
            pt = _pack_rows(HT, 480)               # [G, NUS*W480]
            pt = pt.reshape(NGC, 128, NUS, W480).transpose(2, 1, 0, 3)
            m[f'hutb_{key}'] = np.ascontiguousarray(
                pt.reshape(NUS, 128, NGC * W480))
            dv, de = deg[key]
            dvp = np.ones(UCP, np.float32)
            dvp[:UC] = dv[rows]
            g_idx = np.arange(G)
            u_idx = np.arange(UCP)
            m[f'scde_{key}'] = np.ascontiguousarray(
                (1.0 / (BITVAL[kg] * de))[g_idx].reshape(NGC, 128).T).astype(np.float32)
            scdv = (1.0 / (BITVAL[ku] * dvp))[u_idx].reshape(KU, 128).T
            m[f'scdv_{key}'] = np.ascontiguousarray(scdv).astype(np.float32)
        # x0 user shard
        x0 = np.zeros((UCP, D), np.float32)
        x0[:UC] = user_emb[c * UC:(c + 1) * UC]
        m['x0u'] = np.ascontiguousarray(
            x0.reshape(KU, 128, D).transpose(1, 0, 2)).astype(f8)

        rows = slice(c * GGR, (c + 1) * GGR)
        Hgl = Hg[rows]                             # [512, 4096]
        m['hgb'] = np.ascontiguousarray(
            _pack_rows(Hgl, 512).reshape(KG // 2, 2, 128, GS * W512)
            .transpose(0, 2, 1, 3))
        HTg = Hg[:, rows]                          # [4096, 512] = (Hg^T rows)^T? no:
        # pass-b(g) panel: contraction over g' (full 4096), output = local 512 rows
        # panel[g'percore-chunk partition, local-col] = Hg[local_row, g']^T = Hg.T? we need
        # Hgg^T[g', local g] = Hg[local g, g'] -> HT rows g', cols local -> Hg[rows].T
        HTg = Hgl.T                                # [4096, 512]
        ptg = _pack(HTg, 512)                      # [4096, W512]
        ptg = ptg.reshape(NGC, 128, W512).transpose(1, 0, 2)
        m['hgtb'] = np.ascontiguousarray(ptg.reshape(128, NGC * W512))
        dvg, deg_g = deg['g']
        m['scde_g'] = np.ascontiguousarray(
            (1.0 / (BITVAL[kg] * deg_g)).reshape(NGC, 128).T).astype(np.float32)
        dvgl = dvg[rows]
        m['scdv_g'] = np.ascontiguousarray(
            (1.0 / (BITVAL[kgg] * dvgl)).reshape(KG, 128).T).astype(np.float32)
        m['xg0'] = np.ascontiguousarray(
            group_emb[rows].reshape(KG, 128, D).transpose(1, 0, 2)).astype(f8)

        bid = slice(c * BC, (c + 1) * BC)
        gid = groupid[bid]
        m['gidrow'] = gid[None, :].astype(np.float32)

        m['item_bt'] = np.ascontiguousarray(item_b[bid].T).astype(bf16)
        mlo, mhi = int(starts[c * BC]), int(starts[(c + 1) * BC])
        mid_c = mids[mlo:mhi]
        seg_c = (bseg[mlo:mhi] - c * BC).astype(np.int64)
        Mc = len(mid_c)
        gi = (mid_c // UC) * UCP + (mid_c % UC)
        gi = np.concatenate([gi, np.zeros(MPAD - Mc, np.int64)])
        m['gidx'] = _wrap_idx(gi.astype(np.int16), MPAD)
        segw = np.full((128, NJ), 511, np.int16)
        pos = np.arange(Mc)
        segw[pos % 128, pos // 128] = seg_c
        m['bsegw'] = np.ascontiguousarray(segw)
        segrow = np.full(MPAD, 511.0, np.float32)
        segrow[:Mc] = seg_c
        m['bsegrow'] = segrow[None, :]

        m['w1u'] = att_w1[:D].astype(bf16)
        m['w1i'] = att_w1[D:].astype(bf16)
        m['pw1'] = np.ascontiguousarray(
            pw1.reshape(3, 128, 8).transpose(1, 0, 2).reshape(128, 24)).astype(bf16)
        crow = np.zeros((1, 48), np.float32)
        crow[0, 0:16] = att_b1
        crow[0, 16:32] = att_w2[:, 0]
        crow[0, 32:40] = pb1
        crow[0, 40:48] = pw2[:, 0]
        m['crow'] = crow
        in_maps.append(m)

    meta = dict(MPAD=MPAD, NJ=NJ,
                att_b2=float(inp['att_b2'][0]), pred_b2=float(inp['pred_b2'][0]))
    return in_maps, meta


def _build(meta):
    NJ, MPAD = meta['NJ'], meta['MPAD']
    att_b2, pred_b2 = meta['att_b2'], meta['pred_b2']

    nc = bacc.Bacc("TRN2", target_bir_lowering=False)

    def din(name, shape, dt):
        return nc.dram_tensor(name, list(shape), dt, kind="ExternalInput")

    masks_d = din('masks', (128, 7), U32)
    hub = {k: din(f'hub_{k}', (KU // 2, 128, 2, GS * W512), U32) for k in 'ab'}
    hutb = {k: din(f'hutb_{k}', (NUS, 128, NGC * W480), U32) for k in 'ab'}
    scde = {k: din(f'scde_{k}', (128, NGC), FP32) for k in 'abg'}
    scdv = {k: din(f'scdv_{k}', (128, KU), FP32) for k in 'ab'}
    scdv['g'] = din('scdv_g', (128, KG), FP32)
    x0u = din('x0u', (128, KU, D), F8)
    hgb = din('hgb', (KG // 2, 128, 2, GS * W512), U32)
    hgtb = din('hgtb', (128, NGC * W512), U32)
    xg0 = din('xg0', (128, KG, D), F8)
    gidrow = din('gidrow', (1, B // NC), FP32)
    bsegrow = din('bsegrow', (1, MPAD), FP32)
    item_bt = din('item_bt', (128, 2 * 128), BF16)
    gidx = din('gidx', (16, MPAD // 16), I16)
    bsegw = din('bsegw', (128, NJ), I16)
    w1u = din('w1u', (D, 16), BF16)
    w1i = din('w1i', (D, 16), BF16)
    pw1 = din('pw1', (128, 24), BF16)
    crow = din('crow', (1, 48), FP32)
    out = nc.dram_tensor('out', [BC, 1], FP32, kind="ExternalOutput")

    RG = [list(range(NC))]
    KCH = {'a': KU, 'b': KU, 'g': KG}
    HB = {'g': hgb}
    HB.update(hub)

    with tile.TileContext(nc) as tc:
        with (
            tc.tile_pool(name="pers", bufs=1) as pers,
            tc.tile_pool(name="ps", bufs=1, space="PSUM") as ps,
            tc.tile_pool(name="dram", bufs=1, space="DRAM") as dr,
        ):
            # ---------------- persistent small tiles ----------------
            masks_sb = pers.tile([128, 7], U32, name="masks_sb")
            nc.sync.dma_start(masks_sb[:], masks_d[:])
            w1u_sb = pers.tile([D, 16], BF16, name="w1u_sb")
            nc.sync.dma_start(w1u_sb[:], w1u[:])
            w1i_sb = pers.tile([D, 16], BF16, name="w1i_sb")
            nc.sync.dma_start(w1i_sb[:], w1i[:])
            pw1_sb = pers.tile([128, 3, 8], BF16, name="pw1_sb")
            nc.sync.dma_start(pw1_sb[:], pw1[:].rearrange("p (k o) -> p k o", k=3))
            crow_sb = pers.tile([128, 48], FP32, name="crow_sb")
            nc.sync.dma_start(crow_sb[:], crow[:].to_broadcast([128, 48]))
            crow16 = pers.tile([128, 48], BF16, name="crow16")
            nc.vector.tensor_copy(crow16[:], crow_sb[:])
            ibt_sb = pers.tile([128, 256], BF16, name="ibt_sb")
            nc.sync.dma_start(ibt_sb[:], item_bt[:])
            ident = pers.tile([128, 128], FP32, name="ident")
            make_identity(nc, ident[:])
            ident16 = pers.tile([128, 128], BF16, name="ident16")
            nc.vector.tensor_copy(ident16[:], ident[:])
            ident8 = pers.tile([128, 128], F8, name="ident8")
            nc.vector.tensor_copy(ident8[:], ident[:])
            sc_de, sc_dv, sc_fin = {}, {}, {}
            for k in 'abg':
                sc_de[k] = pers.tile([128, NGC], FP32, name=f"scde{k}")
                nc.sync.dma_start(sc_de[k][:], scde[k][:])
                kc = KCH[k]
                sc_dv[k] = pers.tile([128, kc], FP32, name=f"scdv{k}")
                nc.sync.dma_start(sc_dv[k][:], scdv[k][:])
                if k != 'g':
                    sc_fin[k] = pers.tile([128, kc], FP32, name=f"scfin{k}")
                    nc.vector.tensor_scalar_mul(sc_fin[k][:], sc_dv[k][:], 0.5)

            choose_sb = pers.tile([128, 2, 128], FP32, name="choose_sb")

            # DRAM internals
            ar_in = {(k, it): dr.tile([128, G], BF16, name=f"arin_{k}{it}",
                                      tag=f"arin{k}{it}")
                     for k in 'abg' for it in range(2)}
            ar_out = {(k, it): dr.tile([128, G], BF16, name=f"arout_{k}{it}",
                                       tag=f"arout{k}{it}", addr_space="Shared")
                      for k in 'abg' for it in range(2)}
            x1g_loc = dr.tile([GGR, D], BF16, name="x1g_loc")
            x1g_full = dr.tile([G, D], BF16, name="x1g_full", addr_space="Shared")
            table_loc = dr.tile([UCP, 256], F8, name="table_loc")
            table_full = dr.tile([NC * UCP, 256], F8, name="table_full",
                                 addr_space="Shared")

            # ================= propagation phase =================
            with (
                tc.tile_pool(name="hk_pool", bufs=4) as hkp,
                tc.tile_pool(name="exp_pool", bufs=3) as exp_pool,
                tc.tile_pool(name="panel_pool", bufs=2) as plp,
                tc.tile_pool(name="prop", bufs=2) as prop,
                tc.tile_pool(name="state", bufs=1) as state,
            ):
                x_sb = {
                    'a': state.tile([128, KU, D], F8, name="xa_sb"),
                    'b': state.tile([128, KU, D], F8, name="xb_sb"),
                    'g': state.tile([128, KG, D], F8, name="xg_sb"),
                }
                nc.sync.dma_start(x_sb['a'][:], x0u[:])
                nc.sync.dma_start(x_sb['b'][:], x0u[:])
                nc.sync.dma_start(x_sb['g'][:], xg0[:])
                sn_tiles = {k: state.tile([128, NGC, D], F8, name=f"sn_{k}")
                            for k in 'abg'}
                x1T = {
                    'a': state.tile([128, UCP], BF16, name="x1Ta"),
                    'b': state.tile([128, UCP], BF16, name="x1Tb"),
                    'g': state.tile([128, GGR], BF16, name="x1Tg"),
                }
                ufin = {k: state.tile([128, KU, D], BF16, name=f"ufin_{k}")
                        for k in 'ab'}
                xgf = state.tile([128, KG, D], BF16, name="xgf")
                def expand(bits_ap, nwords, tag, pool):
                    ex = pool.tile([128, nwords, 7], U32, name="ex", tag=tag)
                    nc.vector.tensor_tensor(
                        out=ex[:],
                        in0=bits_ap.unsqueeze(2).to_broadcast([128, nwords, 7]),
                        in1=masks_sb[:].unsqueeze(1).to_broadcast([128, nwords, 7]),
                        op=mybir.AluOpType.bitwise_and)
                    return ex[:].bitcast(F8).rearrange("p w c -> p (w c)")

                def pass_a(mat, it):
                    kp_n = KCH[mat] // 2
                    stage = prop.tile([128, G], BF16, name="stage", tag="stage")
                    for half in range(2):
                        psA = [ps.tile([128, 512], FP32, name=f"pa{s}",
                                       tag=f"psA{s}") for s in range(4)]
                        for kp in range(kp_n):
                            bt = hkp.tile([128, 2, 4 * W512], U32, name="bt",
                                          tag="habits")
                            nc.sync.dma_start(
                                bt[:],
                                HB[mat][kp, :, :,
                                        half * 4 * W512:(half + 1) * 4 * W512])
                            e8 = expand(bt[:].rearrange("p t w -> p (t w)"),
                                        2 * 4 * W512, "haexp", exp_pool)
                            e2 = e8.rearrange("p (t x) -> p t x", t=2)
                            for s in range(4):
                                nc.tensor.matmul(
                                    psA[s][:],
                                    lhsT=x_sb[mat][:, 2 * kp:2 * kp + 2, :],
                                    rhs=e2[:, :, s * E512:s * E512 + 512],
                                    start=(kp == 0), stop=(kp == kp_n - 1),
                                    perf_mode=mybir.MatmulPerfMode.DoubleRow)
                        for s in range(4):
                            gs = half * 4 + s
                            nc.scalar.activation(
                                stage[:, gs * 512:(gs + 1) * 512], psA[s][:],
                                AF.Copy)
                    nc.sync.dma_start(ar_in[(mat, it)][:], stage[:])
                    nc.gpsimd.collective_compute(
                        "AllReduce", mybir.AluOpType.add,
                        ins=[ar_in[(mat, it)].opt()], outs=[ar_out[(mat, it)].opt()],
                        replica_groups=RG)

                def norm_transpose(mat, it):
                    sAR = prop.tile([128, G], BF16, name="sAR", tag="sAR")
                    nc.sync.dma_start(sAR[:], ar_out[(mat, it)][:])
                    for gc in range(NGC):
                        pt = ps.tile([128, 128], BF16, name="pt",
                                     tag=f"psT{gc % 2}")
                        nc.tensor.transpose(
                            pt[:], sAR[:, gc * 128:(gc + 1) * 128], ident16[:])
                        nc.scalar.activation(
                            sn_tiles[mat][:, gc, :], pt[:], AF.Copy,
                            scale=sc_de[mat][:, gc:gc + 1])

                def pass_b(mat, it):
                    DR = mybir.MatmulPerfMode.DoubleRow
                    if mat == 'g':
                        pb_bits = plp.tile([128, NGC * W512], U32, name="pbg",
                                           tag="pbits")
                        nc.sync.dma_start(pb_bits[:], hgtb[:])
                        p8 = expand(pb_bits[:], NGC * W512, "pexp", plp)
                        p2 = p8.rearrange("p (g x) -> p g x", g=NGC)
                        pb = ps.tile([128, 512], FP32, name="pbg_ps", tag="psB0")
                        for gi in range(NGC // 2):
                            nc.tensor.matmul(
                                pb[:], lhsT=sn_tiles['g'][:, 2 * gi:2 * gi + 2, :],
                                rhs=p2[:, 2 * gi:2 * gi + 2, 0:512],
                                start=(gi == 0), stop=(gi == NGC // 2 - 1),
                                perf_mode=DR)
                        nc.vector.tensor_copy(x1T['g'][:], pb[:])
                        return
                    for us in range(NUS):
                        pb_bits = plp.tile([128, NGC * W480], U32, name="pbu",
                                           tag="pbits")
                        nc.sync.dma_start(pb_bits[:], hutb[mat][us])
                        p8 = expand(pb_bits[:], NGC * W480, "pexp", plp)
                        p2 = p8.rearrange("p (g x) -> p g x", g=NGC)
                        pb = ps.tile([128, 512], FP32, name="pb_ps",
                                     tag=f"psB{us % 2}")
                        for gi in range(NGC // 2):
                            nc.tensor.matmul(
                                pb[:, 0:USUB],
                                lhsT=sn_tiles[mat][:, 2 * gi:2 * gi + 2, :],
                                rhs=p2[:, 2 * gi:2 * gi + 2, 0:USUB],
                                start=(gi == 0), stop=(gi == NGC // 2 - 1),
                                perf_mode=DR)
                        nc.scalar.activation(
                            x1T[mat][:, us * USUB:(us + 1) * USUB], pb[:, 0:USUB],
                            AF.Copy)

                def xpose_x(mat, it):
                    last = (it == 1)
                    kch = KCH[mat]
                    if mat == 'g':
                        dst, sc = (xgf, sc_dv['g']) if last else (x_sb['g'], sc_dv['g'])
                    else:
                        dst = ufin[mat] if last else x_sb[mat]
                        sc = sc_fin[mat] if last else sc_dv[mat]
                    for k in range(kch):
                        pt = ps.tile([128, 128], BF16, name="ptx",
                                     tag=f"psT{k % 2}")
                        nc.tensor.transpose(
                            pt[:], x1T[mat][:, k * 128:(k + 1) * 128], ident16[:])
                        nc.scalar.activation(
                            dst[:, k, :], pt[:], AF.Copy, scale=sc[:, k:k + 1])

                upT_r, pTr = {}, {}

                def proj_path(mat):
                    # upT_raw = W1u^T @ x1T_raw  (linear; dv/0.5 scales folded
                    # later per-partition), then transpose to [u, 16] tiles
                    upT_r[mat] = state.tile([16, UCP], BF16, name=f"upTr{mat}")
                    for us in range(NUS):
                        pu = ps.tile([16, USUB], FP32, name="pu", tag="psT1")
                        nc.tensor.matmul(
                            pu[:], lhsT=w1u_sb[:],
                            rhs=x1T[mat][:, us * USUB:(us + 1) * USUB],
                            start=True, stop=True)
                        nc.vector.tensor_copy(
                            upT_r[mat][:, us * USUB:(us + 1) * USUB], pu[:])
                    pTr[mat] = state.tile([128, KU, 16], BF16, name=f"pTr{mat}")
                    for k in range(KU):
                        ptu = ps.tile([128, 16], BF16, name="ptp", tag="psT0")
                        nc.tensor.transpose(
                            ptu[:], upT_r[mat][:, k * 128:(k + 1) * 128],
                            ident16[0:16, 0:16])
                        nc.vector.tensor_copy(pTr[mat][:, k, :], ptu[:])

                for it in range(2):
                    for mat in 'gab':
                        pass_a(mat, it)
                    for mat in 'abg':
                        norm_transpose(mat, it)
                        pass_b(mat, it)
                        xpose_x(mat, it)
                        if it == 1 and mat != 'g':
                            proj_path(mat)

                # final g embedding out + AllGather (1MB)
                nc.sync.dma_start(
                    x1g_loc[:].rearrange("(k p) d -> p k d", p=128), xgf[:])
                nc.gpsimd.collective_compute(
                    "AllGather", mybir.AluOpType.bypass,
                    ins=[x1g_loc.opt()], outs=[x1g_full.opt()],
                    replica_groups=RG)

                # ---------- user combine + table build ----------
                user_t16 = state.tile([128, KU, D], BF16, name="user_t16")
                nc.vector.tensor_add(user_t16[:], ufin['a'][:], ufin['b'][:])
                user_t = state.tile([128, KU, D], F8, name="user_t")
                nc.scalar.activation(user_t[:], user_t16[:], AF.Copy)
                pjA = state.tile([128, KU, 16], BF16, name="pjA")
                nc.vector.tensor_tensor(
                    out=pjA[:], in0=pTr['a'][:],
                    in1=sc_fin['a'][:].unsqueeze(2).to_broadcast([128, KU, 16]),
                    op=mybir.AluOpType.mult)
                pjB = state.tile([128, KU, 16], BF16, name="pjB")
                nc.vector.tensor_tensor(
                    out=pjB[:], in0=pTr['b'][:],
                    in1=sc_fin['b'][:].unsqueeze(2).to_broadcast([128, KU, 16]),
                    op=mybir.AluOpType.mult)
                projT = state.tile([128, KU, 16], F8, name="projT")
                nc.vector.tensor_add(projT[:], pjA[:], pjB[:])
                nc.sync.dma_start(
                    table_loc[:, 0:128].rearrange("(k p) d -> p k d", p=128),
                    user_t[:])
                nc.sync.dma_start(
                    table_loc[:, 128:144].rearrange("(k p) d -> p k d", p=128),
                    projT[:])
                nc.gpsimd.collective_compute(
                    "AllGather", mybir.AluOpType.bypass,
                    ins=[table_loc.opt()], outs=[table_full.opt()],
                    replica_groups=RG)

            # ================= tail =================
            with tc.tile_pool(name="wtp", bufs=1) as wtp:
                wt = wtp.tile([128, NJ, 132], F8, name="wt")
                att_bf = wtp.tile([128, NJ], F8, name="att_bf")

                # ---------- choose via selector one-hot ----------
                with tc.tile_pool(name="chp", bufs=1) as chp:
                    x1g_tiles = chp.tile([128, NGC, D], BF16, name="x1gt")
                    for gc in range(NGC):
                        nc.sync.dma_start(
                            x1g_tiles[:, gc, :],
                            x1g_full[gc * 128:(gc + 1) * 128, :])
                    ones32 = chp.tile([1, 128], FP32, name="ones32")
                    nc.vector.memset(ones32[:], 1.0)
                    gidrow_sb = chp.tile([1, BC], FP32, name="gidrow_sb")
                    nc.sync.dma_start(gidrow_sb[:], gidrow[:])
                    psg = ps.tile([128, BC], FP32, name="psg", tag="psB1")
                    nc.tensor.matmul(psg[:], lhsT=ones32[:], rhs=gidrow_sb[:],
                                     start=True, stop=True)
                    gidbc = chp.tile([128, BC], FP32, name="gidbc")
                    nc.vector.tensor_copy(gidbc[:], psg[:])
                    iotg = chp.tile([128, NGC], I16, name="iotg")
                    nc.gpsimd.iota(iotg[:], pattern=[[128, NGC]], base=0,
                                   channel_multiplier=1)
                    iotgf = chp.tile([128, NGC], FP32, name="iotgf")
                    nc.vector.tensor_copy(iotgf[:], iotg[:])
                    sel_sb = chp.tile([128, 2, NGC, 128], BF16, name="sel_sb")
                    for h in range(2):
                        nc.vector.tensor_tensor(
                            out=sel_sb[:, h],
                            in0=gidbc[:, h * 128:(h + 1) * 128].unsqueeze(1)
                                .to_broadcast([128, NGC, 128]),
                            in1=iotgf[:].unsqueeze(2)
                                .to_broadcast([128, NGC, 128]),
                            op=mybir.AluOpType.is_equal)
                    ps_ch = [ps.tile([128, 128], FP32, name=f"ch{h}",
                                     tag=f"psA{h}") for h in range(2)]
                    for gc in range(NGC):
                        for h in range(2):
                            nc.tensor.matmul(
                                ps_ch[h][:], lhsT=sel_sb[:, h, gc, :],
                                rhs=x1g_tiles[:, gc, :],
                                start=(gc == 0), stop=(gc == NGC - 1))
                    for h in range(2):
                        nc.scalar.activation(choose_sb[:, h, :], ps_ch[h][:],
                                             AF.Copy)

                # ---------- member attention ----------
                with tc.tile_pool(name="tailA", bufs=1) as ta:
                    # segment one-hots built on device
                    iot256 = ta.tile([128, 256], I16, name="iot256")
                    nc.gpsimd.iota(iot256[:], pattern=[[1, 256]], base=0,
                                   channel_multiplier=0)
                    bsegw_sb = ta.tile([128, NJ], I16, name="bsegw_sb")
                    nc.sync.dma_start(bsegw_sb[:], bsegw[:])
                    smb_sb = ta.tile([128, NJ, 2, 128], F8, name="smb_sb")
                    nc.vector.tensor_tensor(
                        out=smb_sb[:].rearrange("p j a b -> p j (a b)"),
                        in0=bsegw_sb[:].unsqueeze(2).to_broadcast([128, NJ, 256]),
                        in1=iot256[:].unsqueeze(1).to_broadcast([128, NJ, 256]),
                        op=mybir.AluOpType.is_equal)
                    onesb = ta.tile([1, 128], FP32, name="onesb")
                    nc.vector.memset(onesb[:], 1.0)
                    bsegrow_sb = ta.tile([1, MPAD], FP32, name="bsegrow_sb")
                    nc.sync.dma_start(bsegrow_sb[:], bsegrow[:])
                    bsegbc = ta.tile([128, MPAD], FP32, name="bsegbc")
                    for cch in range(MPAD // 512):
                        psb = ps.tile([128, 512], FP32, name="psb", tag="psB1")
                        nc.tensor.matmul(
                            psb[:], lhsT=onesb[:],
                            rhs=bsegrow_sb[:, cch * 512:(cch + 1) * 512],
                            start=True, stop=True)
                        nc.scalar.activation(
                            bsegbc[:, cch * 512:(cch + 1) * 512], psb[:], AF.Copy)
                    iotbh = []
                    for h in range(2):
                        it_i = ta.tile([128, NJ], I16, name=f"iotb{h}",
                                       tag=f"iotb{h}")
                        nc.gpsimd.iota(it_i[:], pattern=[[0, NJ]], base=h * 128,
                                       channel_multiplier=1)
                        it_f = ta.tile([128, NJ], FP32, name=f"iotbf{h}",
                                       tag=f"iotbf{h}")
                        nc.vector.tensor_copy(it_f[:], it_i[:])
                        iotbh.append(it_f)
                    sbm_sb = ta.tile([128, NJ, 2, 128], F8, name="sbm_sb")
                    for h in range(2):
                        nc.vector.tensor_tensor(
                            out=sbm_sb[:, :, h, :],
                            in0=bsegbc[:].rearrange("p (j m) -> p j m", j=NJ),
                            in1=iotbh[h][:].unsqueeze(2)
                                .to_broadcast([128, NJ, 128]),
                            op=mybir.AluOpType.is_equal)

                    idx_sb = ta.tile([128, MPAD // 16], I16, name="idx_sb")
                    for rr in range(8):
                        nc.sync.dma_start(
                            idx_sb[rr * 16:(rr + 1) * 16, :], gidx[:])
                    gath = ta.tile([128, NJ, 256], F8, name="gath")
                    nc.gpsimd.dma_gather(
                        out_ap=gath[:], in_ap=table_full[:], idxs_ap=idx_sb[:],
                        num_idxs=MPAD, num_idxs_reg=MPAD, elem_size=256,
                        single_packet=False)

                    iproj = ta.tile([128, 2, 16], F8, name="iproj")
                    for h in range(2):
                        pi = ps.tile([128, 16], FP32, name="pi", tag="psB0")
                        nc.tensor.matmul(pi[:],
                                         lhsT=ibt_sb[:, h * 128:(h + 1) * 128],
                                         rhs=w1i_sb[:], start=True, stop=True)
                        nc.vector.tensor_copy(iproj[:, h, :], pi[:])
                    crow8 = ta.tile([128, 16], F8, name="crow8")
                    nc.vector.tensor_copy(crow8[:], crow_sb[:, 0:16])
                    nc.vector.tensor_tensor(
                        out=iproj[:], in0=iproj[:],
                        in1=crow8[:].unsqueeze(1).to_broadcast([128, 2, 16]),
                        op=mybir.AluOpType.add)

                    ip_all = ta.tile([128, NJ, 16], BF16, name="ip_all")
                    h_all = ta.tile([128, NJ, 16], BF16, name="h_all")
                    hw = ta.tile([128, NJ, 16], FP32, name="hw")
                    logit = ta.tile([128, NJ], FP32, name="logit")
                    att = ta.tile([128, NJ], FP32, name="att")
                    NJH = NJ // 2
                    for q in range(2):
                        jl = slice(q * NJH, (q + 1) * NJH)
                        for j in range(q * NJH, (q + 1) * NJH):
                            pj = ps.tile([128, 16], FP32, name="pj", tag="psB1")
                            nc.tensor.matmul(
                                pj[:], lhsT=sbm_sb[:, j],
                                rhs=iproj[:], start=True, stop=True,
                                perf_mode=mybir.MatmulPerfMode.DoubleRow)
                            nc.vector.tensor_copy(ip_all[:, j, :], pj[:])
                        nc.vector.tensor_add(h_all[:, jl], gath[:, jl, 128:144],
                                             ip_all[:, jl])
                        nc.scalar.activation(h_all[:, jl], h_all[:, jl], AF.Relu)
                        nc.vector.tensor_tensor(
                            out=hw[:, jl], in0=h_all[:, jl],
                            in1=crow16[:, 16:32].unsqueeze(1)
                                .to_broadcast([128, NJH, 16]),
                            op=mybir.AluOpType.mult)
                        nc.vector.reduce_sum(logit[:, jl], hw[:, jl],
                                             axis=mybir.AxisListType.X)
                        nc.scalar.activation(att[:, jl], logit[:, jl], AF.Exp,
                                             bias=att_b2)
                        nc.vector.tensor_copy(att_bf[:, jl], att[:, jl])
                        nc.vector.tensor_tensor(
                            out=wt[:, jl, 0:128], in0=gath[:, jl, 0:128],
                            in1=att_bf[:, jl].unsqueeze(2)
                                .to_broadcast([128, NJH, 128]),
                            op=mybir.AluOpType.mult)
                        nc.vector.tensor_copy(wt[:, jl, 128:129],
                                              att_bf[:, jl].unsqueeze(2))

                with tc.tile_pool(name="tailB", bufs=1) as tb:
                    ps_ag = [ps.tile([128, 129], FP32, name=f"ag{h}",
                                     tag=f"psA{2 + h}") for h in range(2)]
                    for jp in range(NJ // 2):
                        for h in range(2):
                            nc.tensor.matmul(
                                ps_ag[h][:],
                                lhsT=smb_sb[:, 2 * jp:2 * jp + 2, h, :],
                                rhs=wt[:, 2 * jp:2 * jp + 2, 0:129],
                                start=(jp == 0), stop=(jp == NJ // 2 - 1),
                                perf_mode=mybir.MatmulPerfMode.DoubleRow)

                    gT = tb.tile([128, 2, 128], BF16, name="gT")
                    for h in range(2):
                        den_r = tb.tile([128, 1], FP32, name="den_r", tag="den_r")
                        nc.vector.reciprocal(den_r[:], ps_ag[h][:, 128:129])
                        grp = tb.tile([128, 128], FP32, name="grp", tag="grp")
                        nc.vector.tensor_tensor(
                            out=grp[:], in0=ps_ag[h][:, 0:128],
                            in1=den_r[:].to_broadcast([128, 128]),
                            op=mybir.AluOpType.mult)
                        nc.vector.tensor_add(grp[:], grp[:], choose_sb[:, h, :])
                        pt = ps.tile([128, 128], FP32, name="ptg", tag="psB0")
                        nc.tensor.transpose(pt[:], grp[:], ident[:])
                        nc.vector.tensor_copy(gT[:, h, :], pt[:])

                    giT = tb.tile([128, 2, 128], BF16, name="giT")
                    nc.vector.tensor_tensor(
                        out=giT[:], in0=gT[:],
                        in1=ibt_sb[:].rearrange("p (h b) -> p h b", h=2),
                        op=mybir.AluOpType.mult)

                    out_sb = tb.tile([128, 2], FP32, name="out_sb")
                    for h in range(2):
                        pp = ps.tile([128, 8], FP32, name="pp", tag="psB1")
                        ne = [giT[:, h, :], gT[:, h, :],
                              ibt_sb[:, h * 128:(h + 1) * 128]]
                        for kk in range(3):
                            nc.tensor.matmul(pp[:], lhsT=ne[kk],
                                             rhs=pw1_sb[:, kk, :],
                                             start=(kk == 0), stop=(kk == 2))
                        h2 = tb.tile([128, 8], FP32, name="h2", tag="h2")
                        nc.vector.tensor_tensor(
                            out=h2[:], in0=pp[:],
                            in1=crow_sb[:, 32:40],
                            op=mybir.AluOpType.add)
                        nc.scalar.activation(h2[:], h2[:], AF.Relu)
                        nc.vector.tensor_tensor(
                            out=h2[:], in0=h2[:],
                            in1=crow_sb[:, 40:48],
                            op=mybir.AluOpType.mult)
                        l2 = tb.tile([128, 1], FP32, name="l2", tag="l2")
                        nc.vector.reduce_sum(l2[:], h2[:],
                                             axis=mybir.AxisListType.X)
                        nc.scalar.activation(out_sb[:, h:h + 1], l2[:],
                                             AF.Sigmoid, bias=pred_b2)
                    nc.sync.dma_start(
                        out[:].rearrange("(h p) o -> p h o", p=128),
                        out_sb[:].unsqueeze(2))

    nc.finalize()
    return nc


def kernel(**inputs):
    in_maps, meta = _prep(inputs)
    nc = _build(meta)
    res = run_bass_kernel_spmd(nc, in_maps, list(range(NC)))
    outs = [res.results[c]['out'] for c in range(NC)]
    return np.concatenate(outs, axis=0).astype(np.float32)


# revision 23
# speedup vs baseline: 1.0453x; 1.0453x over previous
"""Trainium2 Bass kernel for nn_ModelName_86242943303934 (gnn_message_passing).

Self-contained: takes FULL inputs, shards across 8 NeuronCores internally,
runs one SPMD Bass/Tile program, gathers the full [2048, 1] output.

v2: inputs are bit-packed on host (7 bits/byte for the 0/1 incidence
matrices) and expanded on device with a single uint32 AND per 28 output
columns, bitcast to fp8 (each bit position k yields an exact power-of-two
value BITVAL[k], folded into the 1/de and 1/dv normalization scales).
All transposes run on the tensor engine (PSUM) with the per-partition
de/dv scales fused into the PSUM->SBUF copies on the scalar engine.
Segment one-hot matrices and the groupid selector are built on device
from iota + is_equal; choose_emb comes from a 1MB AllGather of the
propagated group embedding instead of host-gathered H_gg rows.
"""
import sys
sys.path.insert(0, '/opt/trn_rl_repo')

import numpy as np
import ml_dtypes

import concourse.bass as bass
import concourse.mybir as mybir
import concourse.tile as tile
from concourse import bacc
from concourse.bass_utils import run_bass_kernel_spmd
from concourse.masks import make_identity

bf16 = ml_dtypes.bfloat16
f8 = ml_dtypes.float8_e4m3fn
FP32 = mybir.dt.float32
BF16 = mybir.dt.bfloat16
F8 = mybir.dt.float8e4
U32 = mybir.dt.uint32
I16 = mybir.dt.int16

NC = 8
U, G, D, B = 30000, 4096, 128, 2048
UC = U // NC            # 3750 local users
KU = 30                 # user chunks of 128 (padded)
UCP = KU * 128          # 3840
USUB = 480              # pass-B u-subtile width (8 * 480 = 3840)
NUS = 8
GGR = G // NC           # 512 local H_gg rows
KG = 4                  # gg chunks of 128
BC = B // NC            # 256 batch rows per core
NGC = 32                # g chunks of 128
GS = 8                  # 512-col slices of G
W512 = 19               # u32 words per 512-col packed slice
W480 = 18               # u32 words per 480-col packed slice
E512 = W512 * 28        # 532 expanded cols per 512-slice
E480 = W480 * 28        # 504 expanded cols per 480-slice

# value of a set bit k when its byte is reinterpreted as fp8e4m3
BITVAL = np.array([2.0**-9, 2.0**-8, 2.0**-7, 2.0**-6, 2.0**-5, 2.0**-3, 2.0],
                  np.float32)

AF = mybir.ActivationFunctionType


def _slotmap(w):
    """Positions m in [0,w) -> (word j, bit k, byte h) for the packing whose
    on-device expansion (AND with masks, fp8 bitcast) restores column order."""
    nfull, rem = divmod(w, 28)
    assert rem % 4 == 0 and rem // 4 <= 7
    m = np.arange(w)
    j = m // 28
    q = m - 28 * j
    k = q // 4
    h = q % 4
    return j, k, h, nfull + (1 if rem else 0)


def _pack(block, w):
    """block [R, w] of 0/1 -> uint32 words [R, nw]."""
    j, k, h, nw = _slotmap(w)
    bits = (block > 0.5).astype(np.uint32) << (8 * h + k).astype(np.uint32)[None, :]
    starts = np.searchsorted(j, np.arange(nw))
    return np.ascontiguousarray(
        np.bitwise_or.reduceat(bits, starts, axis=1).astype(np.uint32))


def _kcol(w):
    _, k, _, _ = _slotmap(w)
    return k


def _pack_rows(Mrows, w):
    """Mrows [R, C] with C % w == 0 -> [R, (C//w)*nw] u32, per-w-slice packed."""
    R, C = Mrows.shape
    ns = C // w
    outs = [_pack(Mrows[:, s * w:(s + 1) * w], w) for s in range(ns)]
    return np.ascontiguousarray(np.concatenate(outs, axis=1))


def _wrap_idx(idx, n):
    cols = (n + 15) // 16
    w = np.zeros((16, cols), np.int16)
    for i in range(n):
        w[i % 16, i // 16] = idx[i]
    return w


def _prep(inputs):
    inp = {k: np.asarray(v) for k, v in inputs.items()}
    H = {'a': inp['H_ug'].astype(np.float32),
         'b': inp['H_ug_affect'].astype(np.float32)}
    Hg = inp['H_gg'].astype(np.float32)
    user_emb = inp['user_emb'].astype(np.float32)
    group_emb = inp['group_emb'].astype(np.float32)
    item_emb = inp['item_emb'].astype(np.float32)
    groupid = inp['groupid'].astype(np.int64)
    itemid = inp['itemid'].astype(np.int64)
    mids = inp['member_user_ids'].astype(np.int64)
    bseg = inp['batch_seg'].astype(np.int64)

    att_w1 = inp['att_w1'].astype(np.float32)
    att_b1 = inp['att_b1'].astype(np.float32)
    att_w2 = inp['att_w2'].astype(np.float32)
    pw1 = inp['pred_w1'].astype(np.float32)
    pb1 = inp['pred_b1'].astype(np.float32)
    pw2 = inp['pred_w2'].astype(np.float32)

    deg = {}
    for m, Hm in (('a', H['a']), ('b', H['b']), ('g', Hg)):
        deg[m] = (Hm.sum(1) + 1e-5, Hm.sum(0) + 1e-5)

    counts = np.bincount(bseg, minlength=B)
    starts = np.concatenate([[0], np.cumsum(counts)])
    mc = [int(starts[(c + 1) * BC] - starts[c * BC]) for c in range(NC)]
    MPAD = int(-(-max(mc) // 256) * 256)
    NJ = MPAD // 128

    item_b = item_emb[itemid]                      # [B, D] host gather

    kg = np.tile(_kcol(512), GS)                   # k-index per g column
    ku = np.tile(_kcol(480), NUS)                  # k-index per local user col
    kgg = _kcol(512)                               # k-index per local gg col

    masks = (np.uint32(0x01010101) << np.arange(7, dtype=np.uint32))
    masks = np.tile(masks[None, :], (128, 1)).astype(np.uint32)

    in_maps = []
    for c in range(NC):
        m = {'masks': masks}
        for key in ('a', 'b'):
            rows = slice(c * UC, (c + 1) * UC)
            Hp = np.zeros((UCP, G), np.float32)
            Hp[:UC] = H[key][rows]
            # pass-a bits: [KU, 128, GS*W512]
            pb = _pack_rows(Hp, 512)               # [UCP, 152]
            m[f'hub_{key}'] = np.ascontiguousarray(
                pb.reshape(KU // 2, 2, 128, GS * W512).transpose(0, 2, 1, 3))
            # pass-b panel bits: per us, [128(part=g%128), NGC*W480]
            HT = Hp.T                              # [G, UCP]
            pt = _pack_rows(HT, 480)               # [G, NUS*W480]
            pt = pt.reshape(NGC, 128, NUS, W480).transpose(2, 1, 0, 3)
            m[f'hutb_{key}'] = np.ascontiguousarray(
                pt.reshape(NUS, 128, NGC * W480))
            dv, de = deg[key]
            dvp = np.ones(UCP, np.float32)
            dvp[:UC] = dv[rows]
            g_idx = np.arange(G)
            u_idx = np.arange(UCP)
            m[f'scde_{key}'] = np.ascontiguousarray(
                (1.0 / (BITVAL[kg] * de))[g_idx].reshape(NGC, 128).T).astype(np.float32)
            scdv = (1.0 / (BITVAL[ku] * dvp))[u_idx].reshape(KU, 128).T
            m[f'scdv_{key}'] = np.ascontiguousarray(scdv).astype(np.float32)
        # x0 user shard
        x0 = np.zeros((UCP, D), np.float32)
        x0[:UC] = user_emb[c * UC:(c + 1) * UC]
        m['x0u'] = np.ascontiguousarray(
            x0.reshape(KU, 128, D).transpose(1, 0, 2)).astype(f8)

        rows = slice(c * GGR, (c + 1) * GGR)
        Hgl = Hg[rows]                             # [512, 4096]
        m['hgb'] = np.ascontiguousarray(
            _pack_rows(Hgl, 512).reshape(KG // 2, 2, 128, GS * W512)
            .transpose(0, 2, 1, 3))
        HTg = Hg[:, rows]                          # [4096, 512] = (Hg^T rows)^T? no:
        # pass-b(g) panel: contraction over g' (full 4096), output = local 512 rows
        # panel[g'percore-chunk partition, local-col] = Hg[local_row, g']^T = Hg.T? we need
        # Hgg^T[g', local g] = Hg[local g, g'] -> HT rows g', cols local -> Hg[rows].T
        HTg = Hgl.T                                # [4096, 512]
        ptg = _pack(HTg, 512)                      # [4096, W512]
        ptg = ptg.reshape(NGC, 128, W512).transpose(1, 0, 2)
        m['hgtb'] = np.ascontiguousarray(ptg.reshape(128, NGC * W512))
        dvg, deg_g = deg['g']
        m['scde_g'] = np.ascontiguousarray(
            (1.0 / (BITVAL[kg] * deg_g)).reshape(NGC, 128).T).astype(np.float32)
        dvgl = dvg[rows]
        m['scdv_g'] = np.ascontiguousarray(
            (1.0 / (BITVAL[kgg] * dvgl)).reshape(KG, 128).T).astype(np.float32)
        m['xg0'] = np.ascontiguousarray(
            group_emb[rows].reshape(KG, 128, D).transpose(1, 0, 2)).astype(f8)

        bid = slice(c * BC, (c + 1) * BC)
        gid = groupid[bid]
        m['gidrow'] = gid[None, :].astype(np.float32)

        m['item_bt'] = np.ascontiguousarray(item_b[bid].T).astype(bf16)
        mlo, mhi = int(starts[c * BC]), int(starts[(c + 1) * BC])
        mid_c = mids[mlo:mhi]
        seg_c = (bseg[mlo:mhi] - c * BC).astype(np.int64)
        Mc = len(mid_c)
        gi = (mid_c // UC) * UCP + (mid_c % UC)
        gi = np.concatenate([gi, np.zeros(MPAD - Mc, np.int64)])
        m['gidx'] = _wrap_idx(gi.astype(np.int16), MPAD)
        segw = np.full((128, NJ), 511, np.int16)
        pos = np.arange(Mc)
        segw[pos % 128, pos // 128] = seg_c
        m['bsegw'] = np.ascontiguousarray(segw)
        segrow = np.full(MPAD, 511.0, np.float32)
        segrow[:Mc] = seg_c
        m['bsegrow'] = segrow[None, :]

        m['w1u'] = att_w1[:D].astype(bf16)
        m['w1i'] = att_w1[D:].astype(bf16)
        m['pw1'] = np.ascontiguousarray(
            pw1.reshape(3, 128, 8).transpose(1, 0, 2).reshape(128, 24)).astype(bf16)
        crow = np.zeros((1, 48), np.float32)
        crow[0, 0:16] = att_b1
        crow[0, 16:32] = att_w2[:, 0]
        crow[0, 32:40] = pb1
        crow[0, 40:48] = pw2[:, 0]
        m['crow'] = crow
        in_maps.append(m)

    meta = dict(MPAD=MPAD, NJ=NJ,
                att_b2=float(inp['att_b2'][0]), pred_b2=float(inp['pred_b2'][0]))
    return in_maps, meta


def _build(meta):
    NJ, MPAD = meta['NJ'], meta['MPAD']
    att_b2, pred_b2 = meta['att_b2'], meta['pred_b2']

    nc = bacc.Bacc("TRN2", target_bir_lowering=False)

    def din(name, shape, dt):
        return nc.dram_tensor(name, list(shape), dt, kind="ExternalInput")

    masks_d = din('masks', (128, 7), U32)
    hub = {k: din(f'hub_{k}', (KU // 2, 128, 2, GS * W512), U32) for k in 'ab'}
    hutb = {k: din(f'hutb_{k}', (NUS, 128, NGC * W480), U32) for k in 'ab'}
    scde = {k: din(f'scde_{k}', (128, NGC), FP32) for k in 'abg'}
    scdv = {k: din(f'scdv_{k}', (128, KU), FP32) for k in 'ab'}
    scdv['g'] = din('scdv_g', (128, KG), FP32)
    x0u = din('x0u', (128, KU, D), F8)
    hgb = din('hgb', (KG // 2, 128, 2, GS * W512), U32)
    hgtb = din('hgtb', (128, NGC * W512), U32)
    xg0 = din('xg0', (128, KG, D), F8)
    gidrow = din('gidrow', (1, B // NC), FP32)
    bsegrow = din('bsegrow', (1, MPAD), FP32)
    item_bt = din('item_bt', (128, 2 * 128), BF16)
    gidx = din('gidx', (16, MPAD // 16), I16)
    bsegw = din('bsegw', (128, NJ), I16)
    w1u = din('w1u', (D, 16), BF16)
    w1i = din('w1i', (D, 16), BF16)
    pw1 = din('pw1', (128, 24), BF16)
    crow = din('crow', (1, 48), FP32)
    out = nc.dram_tensor('out', [BC, 1], FP32, kind="ExternalOutput")

    RG = [list(range(NC))]
    KCH = {'a': KU, 'b': KU, 'g': KG}
    HB = {'g': hgb}
    HB.update(hub)

    with tile.TileContext(nc) as tc:
        with (
            tc.tile_pool(name="pers", bufs=1) as pers,
            tc.tile_pool(name="ps", bufs=1, space="PSUM") as ps,
            tc.tile_pool(name="dram", bufs=1, space="DRAM") as dr,
        ):
            # ---------------- persistent small tiles ----------------
            masks_sb = pers.tile([128, 7], U32, name="masks_sb")
            nc.sync.dma_start(masks_sb[:], masks_d[:])
            w1u_sb = pers.tile([D, 16], BF16, name="w1u_sb")
            nc.sync.dma_start(w1u_sb[:], w1u[:])
            w1i_sb = pers.tile([D, 16], BF16, name="w1i_sb")
            nc.sync.dma_start(w1i_sb[:], w1i[:])
            pw1_sb = pers.tile([128, 3, 8], BF16, name="pw1_sb")
            nc.sync.dma_start(pw1_sb[:], pw1[:].rearrange("p (k o) -> p k o", k=3))
            crow_sb = pers.tile([128, 48], FP32, name="crow_sb")
            nc.sync.dma_start(crow_sb[:], crow[:].to_broadcast([128, 48]))
            crow16 = pers.tile([128, 48], BF16, name="crow16")
            nc.vector.tensor_copy(crow16[:], crow_sb[:])
            ibt_sb = pers.tile([128, 256], BF16, name="ibt_sb")
            nc.sync.dma_start(ibt_sb[:], item_bt[:])
            ident = pers.tile([128, 128], FP32, name="ident")
            make_identity(nc, ident[:])
            ident16 = pers.tile([128, 128], BF16, name="ident16")
            nc.vector.tensor_copy(ident16[:], ident[:])
            ident8 = pers.tile([128, 128], F8, name="ident8")
            nc.vector.tensor_copy(ident8[:], ident[:])
            sc_de, sc_dv, sc_fin = {}, {}, {}
            for k in 'abg':
                sc_de[k] = pers.tile([128, NGC], FP32, name=f"scde{k}")
                nc.sync.dma_start(sc_de[k][:], scde[k][:])
                kc = KCH[k]
                sc_dv[k] = pers.tile([128, kc], FP32, name=f"scdv{k}")
                nc.sync.dma_start(sc_dv[k][:], scdv[k][:])
                if k != 'g':
                    sc_fin[k] = pers.tile([128, kc], FP32, name=f"scfin{k}")
                    nc.vector.tensor_scalar_mul(sc_fin[k][:], sc_dv[k][:], 0.5)

            choose_sb = pers.tile([128, 2, 128], FP32, name="choose_sb")

            # DRAM internals
            ar_in = {(k, it): dr.tile([128, G], BF16, name=f"arin_{k}{it}",
                                      tag=f"arin{k}{it}")
                     for k in 'abg' for it in range(2)}
            ar_out = {(k, it): dr.tile([128, G], BF16, name=f"arout_{k}{it}",
                                       tag=f"arout{k}{it}", addr_space="Shared")
                      for k in 'abg' for it in range(2)}
            x1g_loc = dr.tile([GGR, D], BF16, name="x1g_loc")
            x1g_full = dr.tile([G, D], BF16, name="x1g_full", addr_space="Shared")
            table_loc = dr.tile([UCP, 256], F8, name="table_loc")
            table_full = dr.tile([NC * UCP, 256], F8, name="table_full",
                                 addr_space="Shared")

            # ================= propagation phase =================
            with (
                tc.tile_pool(name="hk_pool", bufs=4) as hkp,
                tc.tile_pool(name="exp_pool", bufs=3) as exp_pool,
                tc.tile_pool(name="panel_pool", bufs=2) as plp,
                tc.tile_pool(name="prop", bufs=2) as prop,
                tc.tile_pool(name="state", bufs=1) as state,
            ):
                x_sb = {
                    'a': state.tile([128, KU, D], F8, name="xa_sb"),
                    'b': state.tile([128, KU, D], F8, name="xb_sb"),
                    'g': state.tile([128, KG, D], F8, name="xg_sb"),
                }
                nc.sync.dma_start(x_sb['a'][:], x0u[:])
                nc.sync.dma_start(x_sb['b'][:], x0u[:])
                nc.sync.dma_start(x_sb['g'][:], xg0[:])
                sn_tiles = {k: state.tile([128, NGC, D], F8, name=f"sn_{k}")
                            for k in 'abg'}
                x1T = {
                    'a': state.tile([128, UCP], BF16, name="x1Ta"),
                    'b': state.tile([128, UCP], BF16, name="x1Tb"),
                    'g': state.tile([128, GGR], BF16, name="x1Tg"),
                }
                ufin = {k: state.tile([128, KU, D], BF16, name=f"ufin_{k}")
                        for k in 'ab'}
                xgf = state.tile([128, KG, D], BF16, name="xgf")
                def expand(bits_ap, nwords, tag, pool):
                    ex = pool.tile([128, nwords, 7], U32, name="ex", tag=tag)
                    nc.vector.tensor_tensor(
                        out=ex[:],
                        in0=bits_ap.unsqueeze(2).to_broadcast([128, nwords, 7]),
                        in1=masks_sb[:].unsqueeze(1).to_broadcast([128, nwords, 7]),
                        op=mybir.AluOpType.bitwise_and)
                    return ex[:].bitcast(F8).rearrange("p w c -> p (w c)")

                def pass_a(mat, it):
                    kp_n = KCH[mat] // 2
                    stage = prop.tile([128, G], BF16, name="stage", tag="stage")
                    for half in range(2):
                        psA = [ps.tile([128, 512], FP32, name=f"pa{s}",
                                       tag=f"psA{s}") for s in range(4)]
                        for kp in range(kp_n):
                            bt = hkp.tile([128, 2, 4 * W512], U32, name="bt",
                                          tag="habits")
                            nc.sync.dma_start(
                                bt[:],
                                HB[mat][kp, :, :,
                                        half * 4 * W512:(half + 1) * 4 * W512])
                            e8 = expand(bt[:].rearrange("p t w -> p (t w)"),
                                        2 * 4 * W512, "haexp", exp_pool)
                            e2 = e8.rearrange("p (t x) -> p t x", t=2)
                            for s in range(4):
                                nc.tensor.matmul(
                                    psA[s][:],
                                    lhsT=x_sb[mat][:, 2 * kp:2 * kp + 2, :],
                                    rhs=e2[:, :, s * E512:s * E512 + 512],
                                    start=(kp == 0), stop=(kp == kp_n - 1),
                                    perf_mode=mybir.MatmulPerfMode.DoubleRow)
                        for s in range(4):
                            gs = half * 4 + s
                            nc.scalar.activation(
                                stage[:, gs * 512:(gs + 1) * 512], psA[s][:],
                                AF.Copy)
                    nc.sync.dma_start(ar_in[(mat, it)][:], stage[:])
                    nc.gpsimd.collective_compute(
                        "AllReduce", mybir.AluOpType.add,
                        ins=[ar_in[(mat, it)].opt()], outs=[ar_out[(mat, it)].opt()],
                        replica_groups=RG)

                def norm_transpose(mat, it):
                    sAR = prop.tile([128, G], BF16, name="sAR", tag="sAR")
                    nc.sync.dma_start(sAR[:], ar_out[(mat, it)][:])
                    for gc in range(NGC):
                        pt = ps.tile([128, 128], BF16, name="pt",
                                     tag=f"psT{gc % 2}")
                        nc.tensor.transpose(
                            pt[:], sAR[:, gc * 128:(gc + 1) * 128], ident16[:])
                        nc.scalar.activation(
                            sn_tiles[mat][:, gc, :], pt[:], AF.Copy,
                            scale=sc_de[mat][:, gc:gc + 1])

                def pass_b(mat, it):
                    DR = mybir.MatmulPerfMode.DoubleRow
                    if mat == 'g':
                        pb_bits = plp.tile([128, NGC * W512], U32, name="pbg",
                                           tag="pbits")
                        nc.sync.dma_start(pb_bits[:], hgtb[:])
                        p8 = expand(pb_bits[:], NGC * W512, "pexp", plp)
                        p2 = p8.rearrange("p (g x) -> p g x", g=NGC)
                        pb = ps.tile([128, 512], FP32, name="pbg_ps", tag="psB0")
                        for gi in range(NGC // 2):
                            nc.tensor.matmul(
                                pb[:], lhsT=sn_tiles['g'][:, 2 * gi:2 * gi + 2, :],
                                rhs=p2[:, 2 * gi:2 * gi + 2, 0:512],
                                start=(gi == 0), stop=(gi == NGC // 2 - 1),
                                perf_mode=DR)
                        nc.vector.tensor_copy(x1T['g'][:], pb[:])
                        return
                    for us in range(NUS):
                        pb_bits = plp.tile([128, NGC * W480], U32, name="pbu",
                                           tag="pbits")
                        nc.sync.dma_start(pb_bits[:], hutb[mat][us])
                        p8 = expand(pb_bits[:], NGC * W480, "pexp", plp)
                        p2 = p8.rearrange("p (g x) -> p g x", g=NGC)
                        pb = ps.tile([128, 512], FP32, name="pb_ps",
                                     tag=f"psB{us % 2}")
                        for gi in range(NGC // 2):
                            nc.tensor.matmul(
                                pb[:, 0:USUB],
                                lhsT=sn_tiles[mat][:, 2 * gi:2 * gi + 2, :],
                                rhs=p2[:, 2 * gi:2 * gi + 2, 0:USUB],
                                start=(gi == 0), stop=(gi == NGC // 2 - 1),
                                perf_mode=DR)
                        nc.scalar.activation(
                            x1T[mat][:, us * USUB:(us + 1) * USUB], pb[:, 0:USUB],
                            AF.Copy)

                def xpose_x(mat, it):
                    last = (it == 1)
                    kch = KCH[mat]
                    if mat == 'g':
                        dst, sc = (xgf, sc_dv['g']) if last else (x_sb['g'], sc_dv['g'])
                    else:
                        dst = ufin[mat] if last else x_sb[mat]
                        sc = sc_fin[mat] if last else sc_dv[mat]
                    for k in range(kch):
                        pt = ps.tile([128, 128], BF16, name="ptx",
                                     tag=f"psT{k % 2}")
                        nc.tensor.transpose(
                            pt[:], x1T[mat][:, k * 128:(k + 1) * 128], ident16[:])
                        nc.scalar.activation(
                            dst[:, k, :], pt[:], AF.Copy, scale=sc[:, k:k + 1])

                upT_r, pTr = {}, {}

                def proj_path(mat):
                    # upT_raw = W1u^T @ x1T_raw  (linear; dv/0.5 scales folded
                    # later per-partition), then transpose to [u, 16] tiles
                    upT_r[mat] = state.tile([16, UCP], BF16, name=f"upTr{mat}")
                    for us in range(NUS):
                        pu = ps.tile([16, USUB], FP32, name="pu", tag="psT1")
                        nc.tensor.matmul(
                            pu[:], lhsT=w1u_sb[:],
                            rhs=x1T[mat][:, us * USUB:(us + 1) * USUB],
                            start=True, stop=True)
                        nc.vector.tensor_copy(
                            upT_r[mat][:, us * USUB:(us + 1) * USUB], pu[:])
                    pTr[mat] = state.tile([128, KU, 16], BF16, name=f"pTr{mat}")
                    for k in range(KU):
                        ptu = ps.tile([128, 16], BF16, name="ptp", tag="psT0")
                        nc.tensor.transpose(
                            ptu[:], upT_r[mat][:, k * 128:(k + 1) * 128],
                            ident16[0:16, 0:16])
                        nc.vector.tensor_copy(pTr[mat][:, k, :], ptu[:])

                for it in range(2):
                    for mat in 'abg':
                        pass_a(mat, it)
                    for mat in 'abg':
                        norm_transpose(mat, it)
                        pass_b(mat, it)
                        xpose_x(mat, it)
                        if it == 1 and mat != 'g':
                            proj_path(mat)

                # final g embedding out + AllGather (1MB)
                nc.sync.dma_start(
                    x1g_loc[:].rearrange("(k p) d -> p k d", p=128), xgf[:])
                nc.gpsimd.collective_compute(
                    "AllGather", mybir.AluOpType.bypass,
                    ins=[x1g_loc.opt()], outs=[x1g_full.opt()],
                    replica_groups=RG)

                # ---------- user combine + table build ----------
                user_t16 = state.tile([128, KU, D], BF16, name="user_t16")
                nc.vector.tensor_add(user_t16[:], ufin['a'][:], ufin['b'][:])
                user_t = state.tile([128, KU, D], F8, name="user_t")
                nc.scalar.activation(user_t[:], user_t16[:], AF.Copy)
                pjA = state.tile([128, KU, 16], BF16, name="pjA")
                nc.vector.tensor_tensor(
                    out=pjA[:], in0=pTr['a'][:],
                    in1=sc_fin['a'][:].unsqueeze(2).to_broadcast([128, KU, 16]),
                    op=mybir.AluOpType.mult)
                pjB = state.tile([128, KU, 16], BF16, name="pjB")
                nc.vector.tensor_tensor(
                    out=pjB[:], in0=pTr['b'][:],
                    in1=sc_fin['b'][:].unsqueeze(2).to_broadcast([128, KU, 16]),
                    op=mybir.AluOpType.mult)
                projT = state.tile([128, KU, 16], F8, name="projT")
                nc.vector.tensor_add(projT[:], pjA[:], pjB[:])
                nc.sync.dma_start(
                    table_loc[:, 0:128].rearrange("(k p) d -> p k d", p=128),
                    user_t[:])
                nc.sync.dma_start(
                    table_loc[:, 128:144].rearrange("(k p) d -> p k d", p=128),
                    projT[:])
                nc.gpsimd.collective_compute(
                    "AllGather", mybir.AluOpType.bypass,
                    ins=[table_loc.opt()], outs=[table_full.opt()],
                    replica_groups=RG)

            # ================= tail =================
            with tc.tile_pool(name="wtp", bufs=1) as wtp:
                wt = wtp.tile([128, NJ, 132], F8, name="wt")
                att_bf = wtp.tile([128, NJ], F8, name="att_bf")

                # ---------- choose via selector one-hot ----------
                with tc.tile_pool(name="chp", bufs=1) as chp:
                    x1g_tiles = chp.tile([128, NGC, D], BF16, name="x1gt")
                    for gc in range(NGC):
                        nc.sync.dma_start(
                            x1g_tiles[:, gc, :],
                            x1g_full[gc * 128:(gc + 1) * 128, :])
                    ones32 = chp.tile([1, 128], FP32, name="ones32")
                    nc.vector.memset(ones32[:], 1.0)
                    gidrow_sb = chp.tile([1, BC], FP32, name="gidrow_sb")
                    nc.sync.dma_start(gidrow_sb[:], gidrow[:])
                    psg = ps.tile([128, BC], FP32, name="psg", tag="psB1")
                    nc.tensor.matmul(psg[:], lhsT=ones32[:], rhs=gidrow_sb[:],
                                     start=True, stop=True)
                    gidbc = chp.tile([128, BC], FP32, name="gidbc")
                    nc.vector.tensor_copy(gidbc[:], psg[:])
                    iotg = chp.tile([128, NGC], I16, name="iotg")
                    nc.gpsimd.iota(iotg[:], pattern=[[128, NGC]], base=0,
                                   channel_multiplier=1)
                    iotgf = chp.tile([128, NGC], FP32, name="iotgf")
                    nc.vector.tensor_copy(iotgf[:], iotg[:])
                    sel_sb = chp.tile([128, 2, NGC, 128], BF16, name="sel_sb")
                    for h in range(2):
                        nc.vector.tensor_tensor(
                            out=sel_sb[:, h],
                            in0=gidbc[:, h * 128:(h + 1) * 128].unsqueeze(1)
                                .to_broadcast([128, NGC, 128]),
                            in1=iotgf[:].unsqueeze(2)
                                .to_broadcast([128, NGC, 128]),
                            op=mybir.AluOpType.is_equal)
                    ps_ch = [ps.tile([128, 128], FP32, name=f"ch{h}",
                                     tag=f"psA{h}") for h in range(2)]
                    for gc in range(NGC):
                        for h in range(2):
                            nc.tensor.matmul(
                                ps_ch[h][:], lhsT=sel_sb[:, h, gc, :],
                                rhs=x1g_tiles[:, gc, :],
                                start=(gc == 0), stop=(gc == NGC - 1))
                    for h in range(2):
                        nc.scalar.activation(choose_sb[:, h, :], ps_ch[h][:],
                                             AF.Copy)

                # ---------- member attention ----------
                with tc.tile_pool(name="tailA", bufs=1) as ta:
                    # segment one-hots built on device
                    iot256 = ta.tile([128, 256], I16, name="iot256")
                    nc.gpsimd.iota(iot256[:], pattern=[[1, 256]], base=0,
                                   channel_multiplier=0)
                    bsegw_sb = ta.tile([128, NJ], I16, name="bsegw_sb")
                    nc.sync.dma_start(bsegw_sb[:], bsegw[:])
                    smb_sb = ta.tile([128, NJ, 2, 128], F8, name="smb_sb")
                    nc.vector.tensor_tensor(
                        out=smb_sb[:].rearrange("p j a b -> p j (a b)"),
                        in0=bsegw_sb[:].unsqueeze(2).to_broadcast([128, NJ, 256]),
                        in1=iot256[:].unsqueeze(1).to_broadcast([128, NJ, 256]),
                        op=mybir.AluOpType.is_equal)
                    onesb = ta.tile([1, 128], FP32, name="onesb")
                    nc.vector.memset(onesb[:], 1.0)
                    bsegrow_sb = ta.tile([1, MPAD], FP32, name="bsegrow_sb")
                    nc.sync.dma_start(bsegrow_sb[:], bsegrow[:])
                    bsegbc = ta.tile([128, MPAD], FP32, name="bsegbc")
                    for cch in range(MPAD // 512):
                        psb = ps.tile([128, 512], FP32, name="psb", tag="psB1")
                        nc.tensor.matmul(
                            psb[:], lhsT=onesb[:],
                            rhs=bsegrow_sb[:, cch * 512:(cch + 1) * 512],
                            start=True, stop=True)
                        nc.scalar.activation(
                            bsegbc[:, cch * 512:(cch + 1) * 512], psb[:], AF.Copy)
                    iotbh = []
                    for h in range(2):
                        it_i = ta.tile([128, NJ], I16, name=f"iotb{h}",
                                       tag=f"iotb{h}")
                        nc.gpsimd.iota(it_i[:], pattern=[[0, NJ]], base=h * 128,
                                       channel_multiplier=1)
                        it_f = ta.tile([128, NJ], FP32, name=f"iotbf{h}",
                                       tag=f"iotbf{h}")
                        nc.vector.tensor_copy(it_f[:], it_i[:])
                        iotbh.append(it_f)
                    sbm_sb = ta.tile([128, NJ, 2, 128], F8, name="sbm_sb")
                    for h in range(2):
                        nc.vector.tensor_tensor(
                            out=sbm_sb[:, :, h, :],
                            in0=bsegbc[:].rearrange("p (j m) -> p j m", j=NJ),
                            in1=iotbh[h][:].unsqueeze(2)
                                .to_broadcast([128, NJ, 128]),
                            op=mybir.AluOpType.is_equal)

                    idx_sb = ta.tile([128, MPAD // 16], I16, name="idx_sb")
                    for rr in range(8):
                        nc.sync.dma_start(
                            idx_sb[rr * 16:(rr + 1) * 16, :], gidx[:])
                    gath = ta.tile([128, NJ, 256], F8, name="gath")
                    nc.gpsimd.dma_gather(
                        out_ap=gath[:], in_ap=table_full[:], idxs_ap=idx_sb[:],
                        num_idxs=MPAD, num_idxs_reg=MPAD, elem_size=256,
                        single_packet=False)

                    iproj = ta.tile([128, 2, 16], F8, name="iproj")
                    for h in range(2):
                        pi = ps.tile([128, 16], FP32, name="pi", tag="psB0")
                        nc.tensor.matmul(pi[:],
                                         lhsT=ibt_sb[:, h * 128:(h + 1) * 128],
                                         rhs=w1i_sb[:], start=True, stop=True)
                        nc.vector.tensor_copy(iproj[:, h, :], pi[:])
                    crow8 = ta.tile([128, 16], F8, name="crow8")
                    nc.vector.tensor_copy(crow8[:], crow_sb[:, 0:16])
                    nc.vector.tensor_tensor(
                        out=iproj[:], in0=iproj[:],
                        in1=crow8[:].unsqueeze(1).to_broadcast([128, 2, 16]),
                        op=mybir.AluOpType.add)

                    ip_all = ta.tile([128, NJ, 16], BF16, name="ip_all")
                    h_all = ta.tile([128, NJ, 16], BF16, name="h_all")
                    hw = ta.tile([128, NJ, 16], FP32, name="hw")
                    logit = ta.tile([128, NJ], FP32, name="logit")
                    att = ta.tile([128, NJ], FP32, name="att")
                    NJH = NJ // 2
                    for q in range(2):
                        jl = slice(q * NJH, (q + 1) * NJH)
                        for j in range(q * NJH, (q + 1) * NJH):
                            pj = ps.tile([128, 16], FP32, name="pj", tag="psB1")
                            nc.tensor.matmul(
                                pj[:], lhsT=sbm_sb[:, j],
                                rhs=iproj[:], start=True, stop=True,
                                perf_mode=mybir.MatmulPerfMode.DoubleRow)
                            nc.vector.tensor_copy(ip_all[:, j, :], pj[:])
                        nc.vector.tensor_add(h_all[:, jl], gath[:, jl, 128:144],
                                             ip_all[:, jl])
                        nc.scalar.activation(h_all[:, jl], h_all[:, jl], AF.Relu)
                        nc.vector.tensor_tensor(
                            out=hw[:, jl], in0=h_all[:, jl],
                            in1=crow16[:, 16:32].unsqueeze(1)
                                .to_broadcast([128, NJH, 16]),
                            op=mybir.AluOpType.mult)
                        nc.vector.reduce_sum(logit[:, jl], hw[:, jl],
                                             axis=mybir.AxisListType.X)
                        nc.scalar.activation(att[:, jl], logit[:, jl], AF.Exp,
                                             bias=att_b2)
                        nc.vector.tensor_copy(att_bf[:, jl], att[:, jl])
                        nc.vector.tensor_tensor(
                            out=wt[:, jl, 0:128], in0=gath[:, jl, 0:128],
                            in1=att_bf[:, jl].unsqueeze(2)
                                .to_broadcast([128, NJH, 128]),
                            op=mybir.AluOpType.mult)
                        nc.vector.tensor_copy(wt[:, jl, 128:129],
                                              att_bf[:, jl].unsqueeze(2))

                with tc.tile_pool(name="tailB", bufs=1) as tb:
                    ps_ag = [ps.tile([128, 129], FP32, name=f"ag{h}",
                                     tag=f"psA{2 + h}") for h in range(2)]
                    for jp in range(NJ // 2):
                        for h in range(2):
                            nc.tensor.matmul(
                                ps_ag[h][:],
                                lhsT=smb_sb[:, 2 * jp:2 * jp + 2, h, :],
                                rhs=wt[:, 2 * jp:2 * jp + 2, 0:129],
                                start=(jp == 0), stop=(jp == NJ // 2 - 1),
                                perf_mode=mybir.MatmulPerfMode.DoubleRow)

                    gT = tb.tile([128, 2, 128], BF16, name="gT")
                    for h in range(2):
                        den_r = tb.tile([128, 1], FP32, name="den_r", tag="den_r")
                        nc.vector.reciprocal(den_r[:], ps_ag[h][:, 128:129])
                        grp = tb.tile([128, 128], FP32, name="grp", tag="grp")
                        nc.vector.tensor_tensor(
                            out=grp[:], in0=ps_ag[h][:, 0:128],
                            in1=den_r[:].to_broadcast([128, 128]),
                            op=mybir.AluOpType.mult)
                        nc.vector.tensor_add(grp[:], grp[:], choose_sb[:, h, :])
                        pt = ps.tile([128, 128], FP32, name="ptg", tag="psB0")
                        nc.tensor.transpose(pt[:], grp[:], ident[:])
                        nc.vector.tensor_copy(gT[:, h, :], pt[:])

                    giT = tb.tile([128, 2, 128], BF16, name="giT")
                    nc.vector.tensor_tensor(
                        out=giT[:], in0=gT[:],
                        in1=ibt_sb[:].rearrange("p (h b) -> p h b", h=2),
                        op=mybir.AluOpType.mult)

                    out_sb = tb.tile([128, 2], FP32, name="out_sb")
                    for h in range(2):
                        pp = ps.tile([128, 8], FP32, name="pp", tag="psB1")
                        ne = [giT[:, h, :], gT[:, h, :],
                              ibt_sb[:, h * 128:(h + 1) * 128]]
                        for kk in range(3):
                            nc.tensor.matmul(pp[:], lhsT=ne[kk],
                                             rhs=pw1_sb[:, kk, :],
                                             start=(kk == 0), stop=(kk == 2))
                        h2 = tb.tile([128, 8], FP32, name="h2", tag="h2")
                        nc.vector.tensor_tensor(
                            out=h2[:], in0=pp[:],
                            in1=crow_sb[:, 32:40],
                            op=mybir.AluOpType.add)
                        nc.scalar.activation(h2[:], h2[:], AF.Relu)
                        nc.vector.tensor_tensor(
                            out=h2[:], in0=h2[:],
                            in1=crow_sb[:, 40:48],
                            op=mybir.AluOpType.mult)
                        l2 = tb.tile([128, 1], FP32, name="l2", tag="l2")
                        nc.vector.reduce_sum(l2[:], h2[:],
                                             axis=mybir.AxisListType.X)
                        nc.scalar.activation(out_sb[:, h:h + 1], l2[:],
                                             AF.Sigmoid, bias=pred_b2)
                    nc.sync.dma_start(
                        out[:].rearrange("(h p) o -> p h o", p=128),
                        out_sb[:].unsqueeze(2))

    nc.finalize()
    return nc


def kernel(**inputs):
    in_maps, meta = _prep(inputs)
    nc = _build(meta)
    res = run_bass_kernel_spmd(nc, in_maps, list(range(NC)))
    outs = [res.results[c]['out'] for c in range(NC)]
    return np.concatenate(outs, axis=0).astype(np.float32)
